# revision 2
# baseline (speedup 1.0000x reference)
"""GAWA decoder v3: fp8-DoubleRow matmuls + host-side input-gate precompute.

Design (per core, BP=512 batch rows, hidden H=256 split as [p=128, j=2]):
  * Host precomputes x-side gate pre-activations for all T steps
    (char table lookup + eword term + L0 biases) -> DMA'd per step as fp8.
    Also h-init, and the collapsed attention/logit constant ccpb.
  * All h-contraction matmuls run fp8e4m3 with perf_mode=DoubleRow:
    one instruction contracts K=256 (both j-planes) at 0.5 cyc/row.
    Plane-selective fp8 identity DR-matmuls inject SBUF tensors (x, a)
    into psum groups; K=1 DR-matmuls against a ones-row fold biases in.
  * PSUM (8 banks): r/z groups per layer get their own ping-pong tags
    (rz01, rz1x, 2 banks each) so step t+1's L0 matmuls only wait on
    step t's sigmoids, not the whole L1 chain; hn shared (2); u shared
    for u0/u1/logits (2).
  * GRU combine: h' = n + z*(h-n) = zh - (z-1)*n with
    zh = z*h      (GpSimd, reads fp8 h)
    p = (z-1)*n   (DVE scalar_tensor_tensor)
    h'_f8 = zh-p  (DVE, fp8 out, feeds next step's DR matmuls)
    h1'_f16 = zh1-p1 (GpSimd; feeds the f16 logits matmul)
  * Logits stay f16 (fp8 rhs would cost too much accuracy); +cc via f16
    identity matmul into psum; evacuation on ACT (copy to f16, DMA out).
"""

import os
import sys

for _p in ("/opt/trn_rl_repo", "/root/.axon_site/_ro/trn_rl_repo"):
    if os.path.isdir(_p) and _p not in sys.path:
        sys.path.insert(0, _p)

import numpy as np

import concourse.bacc as bacc
import concourse.mybir as mybir
import concourse.tile as tile
from concourse.bass_utils import run_bass_kernel_spmd

B, T, V = 4096, 32, 256
E, CE, H = 768, 64, 256
NCORES = 8
BP = B // NCORES          # 512 batch rows per core
BOS, PAD = 1, 0

F16 = mybir.dt.float16
F32 = mybir.dt.float32
F8 = mybir.dt.float8e4
AF = mybir.ActivationFunctionType
ALU = mybir.AluOpType
DR = mybir.MatmulPerfMode.DoubleRow

NP_F8 = mybir.dt.np(F8)
SC = 16.0   # fp8 psum groups run at 16x to keep residuals out of subnormals

# brow bias slice ids (each a [1, 2, 128] fp8 DR-lhsT: plane0=bias, plane1=0)
BR_HN0 = 0   # 2: b_hh0 n-part per jg
BR_RZ1 = 2   # 4: (b_ih1+b_hh1) r,z per (g,jg)
BR_HN1 = 6   # 2: b_hh1 n-part
BR_U1 = 8    # 2: b_ih1 n-part
NBR = 10

_CACHE = {}


def _build_nc():
    nc = bacc.Bacc("TRN2", target_bir_lowering=False, debug=False,
                   num_devices=NCORES)

    dt = nc.dram_tensor
    xg_d = dt("xg", [T, 128, 6, BP], F8, kind="ExternalInput")
    hinit8_d = dt("hinit8", [128, 2, BP], F8, kind="ExternalInput")
    hinit16_d = dt("hinit16", [128, 2, BP], F16, kind="ExternalInput")
    whh0_d = dt("whh0p", [128, 2, 3 * H], F8, kind="ExternalInput")
    wih1_d = dt("wih1p", [128, 2, 3 * H], F8, kind="ExternalInput")
    whh1_d = dt("whh1p", [128, 2, 3 * H], F8, kind="ExternalInput")
    whh0r_d = dt("whh0r", [128, 2, H], F8, kind="ExternalInput")
    wih1r_d = dt("wih1r", [128, 2, H], F8, kind="ExternalInput")
    whh1r_d = dt("whh1r", [128, 2, H], F8, kind="ExternalInput")
    hinitr_d = dt("hinitr", [128, 2, BP], F8, kind="ExternalInput")
    proj_d = dt("projp", [128, 2, V], F16, kind="ExternalInput")
    ccpb_d = dt("ccpb", [128, 2, BP], F16, kind="ExternalInput")
    identdr_d = dt("identdr", [128, 4, 128], F8, kind="ExternalInput")
    ident16_d = dt("ident16", [128, 128], F16, kind="ExternalInput")
    brow_d = dt("brow", [1, 2 * NBR, 128], F8, kind="ExternalInput")
    ones_d = dt("ones8", [1, 2, BP], F8, kind="ExternalInput")
    out_d = dt("out", [T, 128, 2, BP], F16, kind="ExternalOutput")

    BH = BP // 2  # half-batch: two independent pipelines A (cols 0:BH) / B

    with tile.TileContext(nc) as tc:
        with (
            tc.tile_pool(name="wp", bufs=1) as wp,
            tc.tile_pool(name="xp", bufs=3) as xp,
            tc.tile_pool(name="hp", bufs=3) as hp,
            tc.tile_pool(name="gp", bufs=3) as gp,
            tc.tile_pool(name="lp", bufs=2) as lp,
            tc.tile_pool(name="psp", bufs=1, space="PSUM") as psp,
        ):
            dma = nc.sync.dma_start
            mm = nc.tensor.matmul
            act = nc.scalar.activation

            whh0 = wp.tile([128, 2, 3 * H], F8, tag="whh0")
            dma(whh0[:], whh0_d[:])
            whh0r = wp.tile([128, 2, H], F8, tag="whh0r")
            dma(whh0r[:], whh0r_d[:])
            wih1r = wp.tile([128, 2, H], F8, tag="wih1r")
            dma(wih1r[:], wih1r_d[:])
            whh1r = wp.tile([128, 2, H], F8, tag="whh1r")
            dma(whh1r[:], whh1r_d[:])
            hir = wp.tile([128, 2, BP], F8, tag="hir")
            dma(hir[:], hinitr_d[:])
            wih1 = wp.tile([128, 2, 3 * H], F8, tag="wih1")
            dma(wih1[:], wih1_d[:])
            whh1 = wp.tile([128, 2, 3 * H], F8, tag="whh1")
            dma(whh1[:], whh1_d[:])
            proj = wp.tile([128, 2, V], F16, tag="proj")
            dma(proj[:], proj_d[:])
            ccpb = wp.tile([128, 2, BP], F16, tag="ccpb")
            dma(ccpb[:], ccpb_d[:])
            identdr = wp.tile([128, 4, 128], F8, tag="identdr")
            dma(identdr[:], identdr_d[:])
            ident16 = wp.tile([128, 128], F16, tag="ident16")
            dma(ident16[:], ident16_d[:])
            brow = wp.tile([1, 2 * NBR, 128], F8, tag="brow")
            dma(brow[:], brow_d[:])
            ones8 = wp.tile([1, 2, BP], F8, tag="ones8")
            dma(ones8[:], ones_d[:])
            hi8 = wp.tile([128, 2, BP], F8, tag="hi8")
            dma(hi8[:], hinit8_d[:])
            hi16 = wp.tile([128, 2, BP], F16, tag="hi16")
            dma(hi16[:], hinit16_d[:])

            def brow_mm(ps2d, col, X, stop):
                # += broadcast bias via K=1 DR matmul: [1,2,128] x [1,2,BH]
                mm(ps2d, brow[:, 2 * col:2 * col + 2, :],
                   ones8[:, :, X * BH:(X + 1) * BH],
                   start=False, stop=stop, perf_mode=DR)

            def id_mm(ps2d, jg, rhs3d, start, stop):
                # += plane-jg of rhs via plane-selective fp8 identity
                mm(ps2d, identdr[:, 2 * jg:2 * jg + 2, :], rhs3d,
                   start=start, stop=stop, perf_mode=DR)

            def hb(tile3d, X):
                return tile3d[:, :, X * BH:(X + 1) * BH]

            xg_tiles = {}

            def fetch_xg(t):
                xt = xp.tile([128, 6, BP], F8, tag="xg")
                dma(xt[:], xg_d[t])
                xg_tiles[t] = xt

            fetch_xg(0)
            fetch_xg(1)

            class Half:
                """One batch-half's pipeline (columns X*BH:(X+1)*BH).

                Two strands: the L0 strand runs one step ahead of the L1
                strand, so L0(t+1) overlaps L1(t) and every stage's inputs
                are produced about a full period before they're consumed.
                """

                def __init__(self, X):
                    self.X = X
                    # (f16 state, fp8 matmul input, fp8 residual) per layer
                    self.h0 = (hb(hi16, X), hb(hi8, X), hb(hir, X))
                    self.h0_prev = self.h0     # one step older (L1 strand)
                    self.h1 = (hb(hi16, X), hb(hi8, X), hb(hir, X))
                    self.rzf = [None, None]
                    self.nv = [None, None]
                    self.av = [None, None]

                def sig(self, t, lyr):
                    X = self.X
                    w_h = whh0 if lyr == 0 else whh1
                    h8x = (self.h0 if lyr == 0 else self.h1)[1]
                    rz = psp.tile([128, 4, BH], F32, tag=f"rz{X}",
                                  name=f"rz{lyr}{X}")
                    for s in range(4):
                        g, jg = divmod(s, 2)
                        mm(rz[:, s, :], w_h[:, :, s * 128:(s + 1) * 128],
                           h8x, start=True, stop=False, perf_mode=DR)
                        if lyr == 0:
                            id_mm(rz[:, s, :], jg,
                                  hb(xg_tiles[t], X)[:, 2 * g:2 * g + 2, :],
                                  False, True)
                        else:
                            mm(rz[:, s, :], wih1[:, :, s * 128:(s + 1) * 128],
                               self.h0_prev[1], start=False, stop=False,
                               perf_mode=DR)
                            brow_mm(rz[:, s, :], BR_RZ1 + s, X, stop=True)
                    rzf = gp.tile([128, 4, BH], F16, tag=f"rzf{lyr}{X}",
                                  name=f"rzf{lyr}{X}")
                    act(rzf[:], rz[:], AF.Sigmoid, scale=1.0 / SC)
                    self.rzf[lyr] = rzf

                def amul(self, t, lyr):
                    X = self.X
                    w_h = whh0 if lyr == 0 else whh1
                    w_r = whh0r if lyr == 0 else whh1r
                    _, h8x, hrx = self.h0 if lyr == 0 else self.h1
                    br_hn = BR_HN0 if lyr == 0 else BR_HN1
                    hn = psp.tile([128, 2, BH], F32, tag=f"hn{X}",
                                  name=f"hn{lyr}{X}")
                    for jg in range(2):
                        mm(hn[:, jg, :],
                           w_h[:, :, (4 + jg) * 128:(5 + jg) * 128],
                           h8x, start=True, stop=False, perf_mode=DR)
                        mm(hn[:, jg, :],
                           w_r[:, :, jg * 128:(jg + 1) * 128],
                           h8x, start=False, stop=False, perf_mode=DR)
                        mm(hn[:, jg, :],
                           w_h[:, :, (4 + jg) * 128:(5 + jg) * 128],
                           hrx, start=False, stop=False, perf_mode=DR)
                        brow_mm(hn[:, jg, :], br_hn + jg, X, stop=True)
                    av = gp.tile([128, 2, BH], F8, tag=f"a{lyr}{X}",
                                 name=f"a{lyr}{X}")
                    nc.vector.tensor_mul(av[:], self.rzf[lyr][:, 0:2, :],
                                         hn[:])
                    self.av[lyr] = av

                def utanh(self, t, lyr):
                    X = self.X
                    ug = psp.tile([128, 2, BH], F32, tag=f"u{X}",
                                  name=f"u{lyr}{X}")
                    for jg in range(2):
                        id_mm(ug[:, jg, :], jg, self.av[lyr][:], True, False)
                        if lyr == 0:
                            id_mm(ug[:, jg, :], jg,
                                  hb(xg_tiles[t], X)[:, 4:6, :], False, True)
                        else:
                            _, h8p, hrp = self.h0_prev
                            mm(ug[:, jg, :],
                               wih1[:, :, (4 + jg) * 128:(5 + jg) * 128],
                               h8p, start=False, stop=False, perf_mode=DR)
                            mm(ug[:, jg, :],
                               wih1r[:, :, jg * 128:(jg + 1) * 128],
                               h8p, start=False, stop=False, perf_mode=DR)
                            mm(ug[:, jg, :],
                               wih1[:, :, (4 + jg) * 128:(5 + jg) * 128],
                               hrp, start=False, stop=False, perf_mode=DR)
                            brow_mm(ug[:, jg, :], BR_U1 + jg, X, stop=True)
                    nv = gp.tile([128, 2, BH], F16, tag=f"n{lyr}{X}",
                                 name=f"n{lyr}{X}")
                    act(nv[:], ug[:], AF.Tanh, scale=1.0 / SC)
                    self.nv[lyr] = nv

                def combine(self, t, lyr):
                    X = self.X
                    zfx = self.rzf[lyr][:, 2:4, :]
                    nv = self.nv[lyr]
                    h16x = (self.h0 if lyr == 0 else self.h1)[0]
                    d = gp.tile([128, 2, BH], F16, tag=f"d{lyr}{X}",
                                name=f"d{lyr}{X}")
                    nc.vector.tensor_sub(d[:], h16x, nv[:])
                    e = gp.tile([128, 2, BH], F16, tag=f"e{lyr}{X}",
                                name=f"e{lyr}{X}")
                    nc.vector.tensor_mul(e[:], d[:], zfx)
                    h16n = hp.tile([128, 2, BH], F16, tag=f"h{lyr}_16{X}",
                                   name=f"h{lyr}n16{X}")
                    nc.vector.tensor_add(h16n[:], e[:], nv[:])
                    h8n = hp.tile([128, 2, BH], F8, tag=f"h{lyr}_8{X}",
                                  name=f"h{lyr}n8{X}")
                    nc.gpsimd.tensor_copy(h8n[:], h16n[:])
                    hrn = hp.tile([128, 2, BH], F8, tag=f"h{lyr}_r{X}",
                                  name=f"h{lyr}nr{X}")
                    # residual: split across engines (DVE for layer 0, GpSimd
                    # for layer 1) to balance load
                    if lyr == 0:
                        nc.vector.tensor_sub(hrn[:], h16n[:], h8n[:])
                    else:
                        nc.gpsimd.tensor_sub(hrn[:], h16n[:], h8n[:])
                    hnew = (h16n, h8n, hrn)
                    if lyr == 0:
                        self.h0_prev = self.h0
                        self.h0 = hnew
                    else:
                        self.h1 = hnew

                def logits(self, t):
                    X = self.X
                    lo = psp.tile([128, 2, BH], F32, tag=f"u{X}", name="lo")
                    for vj in range(2):
                        for j in range(2):
                            mm(lo[:, vj, :],
                               proj[:, j, vj * 128:(vj + 1) * 128],
                               self.h1[0][:, j, :], start=(j == 0),
                               stop=False)
                        mm(lo[:, vj, :], ident16[:],
                           ccpb[:, vj, X * BH:(X + 1) * BH],
                           start=False, stop=True)
                    lo_sb = lp.tile([128, 2, BH], F16, tag=f"lo{X}",
                                    name=f"lo{X}")
                    act(lo_sb[:], lo[:], AF.Copy)
                    dma(out_d[t][:, :, X * BH:(X + 1) * BH], lo_sb[:])

            A, Bh = Half(0), Half(1)

            # 4-strand modulo schedule: per iteration t each half advances
            # its L0 strand to step t+1 and its L1 strand to step t (B's
            # slots sit half an iteration behind A's), so every engine's
            # in-order stream cycles through four independent chains.
            def stage(h, fn_name, t, lyr=None):
                if t < 0 or t >= T:
                    return
                f = getattr(h, fn_name)
                if lyr is None:
                    f(t)
                else:
                    f(t, lyr)

            for t in range(-1, T):
                if t + 3 < T:
                    fetch_xg(t + 3)
                stage(A, "sig", t + 1, 0)
                stage(Bh, "sig", t - 1, 1)
                stage(A, "amul", t + 1, 0)
                stage(Bh, "amul", t - 1, 1)
                stage(A, "utanh", t + 1, 0)
                stage(Bh, "utanh", t - 1, 1)
                if t + 1 < T:
                    A.combine(t + 1, 0)
                elif t + 1 == T:
                    A.h0_prev = A.h0
                stage(Bh, "combine", t - 1, 1)
                stage(Bh, "logits", t - 1)
                stage(A, "sig", t, 1)
                stage(Bh, "sig", t + 1, 0)
                stage(A, "amul", t, 1)
                stage(Bh, "amul", t + 1, 0)
                stage(A, "utanh", t, 1)
                stage(Bh, "utanh", t + 1, 0)
                stage(A, "combine", t, 1)
                stage(A, "logits", t)
                if t + 1 < T:
                    Bh.combine(t + 1, 0)
                elif t + 1 == T:
                    Bh.h0_prev = Bh.h0
                if 0 <= t - 1:
                    xg_tiles.pop(t - 1, None)

            stage(Bh, "sig", T - 1, 1)
            stage(Bh, "amul", T - 1, 1)
            stage(Bh, "utanh", T - 1, 1)
            stage(Bh, "combine", T - 1, 1)
            stage(Bh, "logits", T - 1)

    nc.compile()
    return nc


def _host_prep(inputs):
    """All-numpy precompute; returns per-core input maps."""
    f16 = np.float16
    eword = np.asarray(inputs["eword"], np.float32)
    target_ids = np.asarray(inputs["target_ids"])
    char_emb = np.asarray(inputs["char_emb"], np.float32)
    w_ih0 = np.asarray(inputs["gru_w_ih0"], np.float32)
    w_hh0 = np.asarray(inputs["gru_w_hh0"], np.float32)
    b_ih0 = np.asarray(inputs["gru_b_ih0"], np.float32)
    b_hh0 = np.asarray(inputs["gru_b_hh0"], np.float32)
    w_ih1 = np.asarray(inputs["gru_w_ih1"], np.float32)
    w_hh1 = np.asarray(inputs["gru_w_hh1"], np.float32)
    b_ih1 = np.asarray(inputs["gru_b_ih1"], np.float32)
    b_hh1 = np.asarray(inputs["gru_b_hh1"], np.float32)
    attn_in_w = np.asarray(inputs["attn_in_w"], np.float32)
    attn_in_b = np.asarray(inputs["attn_in_b"], np.float32)
    attn_out_w = np.asarray(inputs["attn_out_w"], np.float32)
    attn_out_b = np.asarray(inputs["attn_out_b"], np.float32)
    eword_proj_w = np.asarray(inputs["eword_proj_w"], np.float32)
    eword_proj_b = np.asarray(inputs["eword_proj_b"], np.float32)
    val_w = np.asarray(inputs["val_w"], np.float32)
    val_b = np.asarray(inputs["val_b"], np.float32)
    proj_w = np.asarray(inputs["proj_w"], np.float32)
    proj_b = np.asarray(inputs["proj_b"], np.float32)

    # x-side gate preactivations for all steps, L0 biases folded in
    in_ids = np.concatenate(
        [np.full((B, 1), BOS, target_ids.dtype), target_ids[:, :-1]], axis=1)
    cemb = char_emb.copy()
    cemb[PAD] = 0.0
    table = cemb @ w_ih0[:, :CE].T                      # (V, 3H)
    ge = eword @ w_ih0[:, CE:].T                        # (B, 3H)
    bias_vec = b_ih0.copy()
    bias_vec[:2 * H] += b_hh0[:2 * H]                   # r,z only
    xg = table[in_ids] + (ge + bias_vec)[:, None, :]    # (B, T, 3H)

    h0 = np.tanh(eword @ eword_proj_w.T + eword_proj_b)  # (B, H)

    wv = attn_in_w[2 * H:3 * H]
    bv = attn_in_b[2 * H:3 * H]
    ev = (eword @ val_w.T + val_b) @ wv.T + bv
    ao = ev @ attn_out_w.T + attn_out_b
    cc = ao @ proj_w.T + proj_b                          # (B, V)

    def pack_w(w):
        # (3H, H) -> lhsT [p_in, j_in, (g,jg)*128+p_out]
        w4 = w.reshape(3, 2, 128, 2, 128)   # g, jg_out, p_out, j_in, p_in
        return np.ascontiguousarray(
            w4.transpose(4, 3, 0, 1, 2).reshape(128, 2, 3 * H))

    def pack8(w):
        a = (SC * pack_w(w)).astype(NP_F8)
        r = ((SC * pack_w(w)) - a.astype(np.float32)).astype(NP_F8)
        return a, r[:, :, 2 * H:]          # residual kept for n-cols only

    whh0p, whh0r = pack8(w_hh0)
    wih1p, wih1r = pack8(w_ih1)
    whh1p, whh1r = pack8(w_hh1)
    shared = {
        "whh0p": whh0p, "whh0r": whh0r,
        "wih1p": wih1p, "wih1r": wih1r,
        "whh1p": whh1p, "whh1r": whh1r,
        "projp": np.ascontiguousarray(
            proj_w.reshape(2, 128, 2, 128).transpose(3, 2, 0, 1)
            .reshape(128, 2, V)).astype(f16),
        "ident16": np.eye(128, dtype=f16),
        "ones8": np.ones((1, 2, BP), np.float32).astype(NP_F8),
    }
    idr = np.zeros((128, 4, 128), np.float32)
    idr[:, 0, :] = np.eye(128)
    idr[:, 3, :] = np.eye(128)
    shared["identdr"] = idr.astype(NP_F8)

    brow = np.zeros((1, 2 * NBR, 128), np.float32)  # values stored at SC x
    b1 = b_ih1 + b_hh1
    for jg in range(2):
        brow[0, 2 * (BR_HN0 + jg)] = \
            b_hh0[2 * H + jg * 128: 2 * H + (jg + 1) * 128]
        brow[0, 2 * (BR_HN1 + jg)] = \
            b_hh1[2 * H + jg * 128: 2 * H + (jg + 1) * 128]
        brow[0, 2 * (BR_U1 + jg)] = \
            b_ih1[2 * H + jg * 128: 2 * H + (jg + 1) * 128]
    for s in range(4):
        g, jg = divmod(s, 2)
        brow[0, 2 * (BR_RZ1 + s)] = \
            b1[g * H + jg * 128: g * H + (jg + 1) * 128]
    shared["brow"] = (SC * brow).astype(NP_F8)

    def jmaj(x2d):
        # (BP, 256) -> [128, 2, BP]
        return np.ascontiguousarray(
            x2d.reshape(BP, 2, 128).transpose(2, 1, 0))

    def per_core(c):
        sl = slice(c * BP, (c + 1) * BP)
        m = dict(shared)
        hj = jmaj(h0[sl]).astype(f16)
        m["hinit16"] = hj
        h8 = hj.astype(np.float32).astype(NP_F8)
        m["hinit8"] = h8
        m["hinitr"] = (hj.astype(np.float32)
                       - h8.astype(np.float32)).astype(NP_F8)
        m["ccpb"] = jmaj(cc[sl]).astype(f16)
        # xg: (BP, T, 3H) -> [T, 128, (g,jg)6, BP], stored at SC x
        xc = xg[sl].reshape(BP, T, 3, 2, 128).transpose(1, 4, 2, 3, 0)
        m["xg"] = np.ascontiguousarray(
            SC * xc.reshape(T, 128, 6, BP)).astype(NP_F8)
        return m

    return [per_core(c) for c in range(NCORES)]


def kernel(**inputs):
    in_maps = _host_prep(inputs)
    if "nc" not in _CACHE:
        _CACHE["nc"] = _build_nc()
    nc = _CACHE["nc"]
    res = run_bass_kernel_spmd(nc, in_maps, list(range(NCORES)),
                               trace=bool(os.environ.get("BASS_TRACE")))
    _CACHE["last_res"] = res
    _CACHE["last_in_maps"] = in_maps
    outs = []
    for c in range(NCORES):
        o = res.results[c]["out"]            # (T, 128, 2, BP) f16
        outs.append(np.ascontiguousarray(
            o.transpose(3, 0, 2, 1).reshape(BP, T, V)).astype(np.float32))
    return np.concatenate(outs, axis=0)


# revision 3
# speedup vs baseline: 1.1305x; 1.1305x over previous
"""GAWA decoder v3: fp8-DoubleRow matmuls + host-side input-gate precompute.

Design (per core, BP=512 batch rows, hidden H=256 split as [p=128, j=2]):
  * Host precomputes x-side gate pre-activations for all T steps
    (char table lookup + eword term + L0 biases) -> DMA'd per step as fp8.
    Also h-init, and the collapsed attention/logit constant ccpb.
  * All h-contraction matmuls run fp8e4m3 with perf_mode=DoubleRow:
    one instruction contracts K=256 (both j-planes) at 0.5 cyc/row.
    Plane-selective fp8 identity DR-matmuls inject SBUF tensors (x, a)
    into psum groups; K=1 DR-matmuls against a ones-row fold biases in.
  * PSUM (8 banks): r/z groups per layer get their own ping-pong tags
    (rz01, rz1x, 2 banks each) so step t+1's L0 matmuls only wait on
    step t's sigmoids, not the whole L1 chain; hn shared (2); u shared
    for u0/u1/logits (2).
  * GRU combine: h' = n + z*(h-n) = zh - (z-1)*n with
    zh = z*h      (GpSimd, reads fp8 h)
    p = (z-1)*n   (DVE scalar_tensor_tensor)
    h'_f8 = zh-p  (DVE, fp8 out, feeds next step's DR matmuls)
    h1'_f16 = zh1-p1 (GpSimd; feeds the f16 logits matmul)
  * Logits stay f16 (fp8 rhs would cost too much accuracy); +cc via f16
    identity matmul into psum; evacuation on ACT (copy to f16, DMA out).
"""

import os
import sys

for _p in ("/opt/trn_rl_repo", "/root/.axon_site/_ro/trn_rl_repo"):
    if os.path.isdir(_p) and _p not in sys.path:
        sys.path.insert(0, _p)

import numpy as np

import concourse.bacc as bacc
import concourse.mybir as mybir
import concourse.tile as tile
from concourse.bass_utils import run_bass_kernel_spmd

B, T, V = 4096, 32, 256
E, CE, H = 768, 64, 256
NCORES = 8
BP = B // NCORES          # 512 batch rows per core
BOS, PAD = 1, 0

F16 = mybir.dt.float16
F32 = mybir.dt.float32
F8 = mybir.dt.float8e4
AF = mybir.ActivationFunctionType
ALU = mybir.AluOpType
DR = mybir.MatmulPerfMode.DoubleRow

NP_F8 = mybir.dt.np(F8)
SC = 16.0   # fp8 psum groups run at 16x to keep residuals out of subnormals

# brow bias slice ids (each a [1, 2, 128] fp8 DR-lhsT: plane0=bias, plane1=0)
BR_HN0 = 0   # 2: b_hh0 n-part per jg
BR_RZ1 = 2   # 4: (b_ih1+b_hh1) r,z per (g,jg)
BR_HN1 = 6   # 2: b_hh1 n-part
BR_U1 = 8    # 2: b_ih1 n-part
NBR = 10

_CACHE = {}


def _build_nc():
    nc = bacc.Bacc("TRN2", target_bir_lowering=False, debug=False,
                   num_devices=NCORES)

    dt = nc.dram_tensor
    xg_d = dt("xg", [T, 128, 6, BP], F8, kind="ExternalInput")
    hinit8_d = dt("hinit8", [128, 2, BP], F8, kind="ExternalInput")
    hinit16_d = dt("hinit16", [128, 2, BP], F16, kind="ExternalInput")
    whh0_d = dt("whh0p", [128, 2, 3 * H], F8, kind="ExternalInput")
    wih1_d = dt("wih1p", [128, 2, 3 * H], F8, kind="ExternalInput")
    whh1_d = dt("whh1p", [128, 2, 3 * H], F8, kind="ExternalInput")
    whh0r_d = dt("whh0r", [128, 2, H], F8, kind="ExternalInput")
    wih1r_d = dt("wih1r", [128, 2, H], F8, kind="ExternalInput")
    whh1r_d = dt("whh1r", [128, 2, H], F8, kind="ExternalInput")
    hinitr_d = dt("hinitr", [128, 2, BP], F8, kind="ExternalInput")
    proj_d = dt("projp", [128, 2, V], F16, kind="ExternalInput")
    ccpb_d = dt("ccpb", [128, 2, BP], F16, kind="ExternalInput")
    identdr_d = dt("identdr", [128, 4, 128], F8, kind="ExternalInput")
    ident16_d = dt("ident16", [128, 128], F16, kind="ExternalInput")
    brow_d = dt("brow", [1, 2 * NBR, 128], F8, kind="ExternalInput")
    ones_d = dt("ones8", [1, 2, BP], F8, kind="ExternalInput")
    out_d = dt("out", [T, 128, 2, BP], F16, kind="ExternalOutput")

    BH = BP // 2  # half-batch: two independent pipelines A (cols 0:BH) / B

    with tile.TileContext(nc) as tc:
        with (
            tc.tile_pool(name="wp", bufs=1) as wp,
            tc.tile_pool(name="xp", bufs=3) as xp,
            tc.tile_pool(name="hp", bufs=3) as hp,
            tc.tile_pool(name="gp", bufs=3) as gp,
            tc.tile_pool(name="lp", bufs=2) as lp,
            tc.tile_pool(name="psp", bufs=1, space="PSUM") as psp,
        ):
            dma = nc.sync.dma_start
            mm = nc.tensor.matmul
            act = nc.scalar.activation

            whh0 = wp.tile([128, 2, 3 * H], F8, tag="whh0")
            dma(whh0[:], whh0_d[:])
            whh0r = wp.tile([128, 2, H], F8, tag="whh0r")
            dma(whh0r[:], whh0r_d[:])
            wih1r = wp.tile([128, 2, H], F8, tag="wih1r")
            dma(wih1r[:], wih1r_d[:])
            whh1r = wp.tile([128, 2, H], F8, tag="whh1r")
            dma(whh1r[:], whh1r_d[:])
            hir = wp.tile([128, 2, BP], F8, tag="hir")
            dma(hir[:], hinitr_d[:])
            wih1 = wp.tile([128, 2, 3 * H], F8, tag="wih1")
            dma(wih1[:], wih1_d[:])
            whh1 = wp.tile([128, 2, 3 * H], F8, tag="whh1")
            dma(whh1[:], whh1_d[:])
            proj = wp.tile([128, 2, V], F16, tag="proj")
            dma(proj[:], proj_d[:])
            ccpb = wp.tile([128, 2, BP], F16, tag="ccpb")
            dma(ccpb[:], ccpb_d[:])
            identdr = wp.tile([128, 4, 128], F8, tag="identdr")
            dma(identdr[:], identdr_d[:])
            ident16 = wp.tile([128, 128], F16, tag="ident16")
            dma(ident16[:], ident16_d[:])
            brow = wp.tile([1, 2 * NBR, 128], F8, tag="brow")
            dma(brow[:], brow_d[:])
            ones8 = wp.tile([1, 2, BP], F8, tag="ones8")
            dma(ones8[:], ones_d[:])
            hi8 = wp.tile([128, 2, BP], F8, tag="hi8")
            dma(hi8[:], hinit8_d[:])
            hi16 = wp.tile([128, 2, BP], F16, tag="hi16")
            dma(hi16[:], hinit16_d[:])

            def brow_mm(ps2d, col, X, stop):
                # += broadcast bias via K=1 DR matmul: [1,2,128] x [1,2,BH]
                mm(ps2d, brow[:, 2 * col:2 * col + 2, :],
                   ones8[:, :, X * BH:(X + 1) * BH],
                   start=False, stop=stop, perf_mode=DR)

            def id_mm(ps2d, jg, rhs3d, start, stop):
                # += plane-jg of rhs via plane-selective fp8 identity
                mm(ps2d, identdr[:, 2 * jg:2 * jg + 2, :], rhs3d,
                   start=start, stop=stop, perf_mode=DR)

            def hb(tile3d, X):
                return tile3d[:, :, X * BH:(X + 1) * BH]

            xg_tiles = {}

            def fetch_xg(t):
                xt = xp.tile([128, 6, BP], F8, tag="xg")
                dma(xt[:], xg_d[t])
                xg_tiles[t] = xt

            fetch_xg(0)
            fetch_xg(1)

            class Half:
                """One batch-half's pipeline (columns X*BH:(X+1)*BH).

                Two strands: the L0 strand runs one step ahead of the L1
                strand, so L0(t+1) overlaps L1(t) and every stage's inputs
                are produced about a full period before they're consumed.
                """

                def __init__(self, X):
                    self.X = X
                    # (f16 state, fp8 matmul input, fp8 residual) per layer
                    self.h0 = (hb(hi16, X), hb(hi8, X), hb(hir, X))
                    self.h0_prev = self.h0     # one step older (L1 strand)
                    self.h1 = (hb(hi16, X), hb(hi8, X), hb(hir, X))
                    self.rzf = [None, None]
                    self.nv = [None, None]
                    self.av = [None, None]

                def sig(self, t, lyr):
                    X = self.X
                    w_h = whh0 if lyr == 0 else whh1
                    h8x = (self.h0 if lyr == 0 else self.h1)[1]
                    rz = psp.tile([128, 4, BH], F32, tag=f"rz{X}",
                                  name=f"rz{lyr}{X}")
                    for s in range(4):
                        g, jg = divmod(s, 2)
                        mm(rz[:, s, :], w_h[:, :, s * 128:(s + 1) * 128],
                           h8x, start=True, stop=False, perf_mode=DR)
                        if lyr == 0:
                            id_mm(rz[:, s, :], jg,
                                  hb(xg_tiles[t], X)[:, 2 * g:2 * g + 2, :],
                                  False, True)
                        else:
                            mm(rz[:, s, :], wih1[:, :, s * 128:(s + 1) * 128],
                               self.h0_prev[1], start=False, stop=False,
                               perf_mode=DR)
                            brow_mm(rz[:, s, :], BR_RZ1 + s, X, stop=True)
                    rzf = gp.tile([128, 4, BH], F16, tag=f"rzf{lyr}{X}",
                                  name=f"rzf{lyr}{X}")
                    act(rzf[:], rz[:], AF.Sigmoid, scale=1.0 / SC)
                    self.rzf[lyr] = rzf
                    # hn matmuls emitted here: PE runs rz+hn back to back and
                    # the psum group is ready well before amul's multiply
                    w_r = whh0r if lyr == 0 else whh1r
                    _, h8i, hri = self.h0 if lyr == 0 else self.h1
                    br_hn = BR_HN0 if lyr == 0 else BR_HN1
                    hn = psp.tile([128, 2, BH], F32, tag=f"hn{X}",
                                  name=f"hn{lyr}{X}")
                    for jg in range(2):
                        mm(hn[:, jg, :],
                           w_h[:, :, (4 + jg) * 128:(5 + jg) * 128],
                           h8i, start=True, stop=False, perf_mode=DR)
                        mm(hn[:, jg, :],
                           w_r[:, :, jg * 128:(jg + 1) * 128],
                           h8i, start=False, stop=False, perf_mode=DR)
                        mm(hn[:, jg, :],
                           w_h[:, :, (4 + jg) * 128:(5 + jg) * 128],
                           hri, start=False, stop=False, perf_mode=DR)
                        brow_mm(hn[:, jg, :], br_hn + jg, X, stop=True)
                    self.hn = hn

                def amul(self, t, lyr):
                    X = self.X
                    av = gp.tile([128, 2, BH], F8, tag=f"a{lyr}{X}",
                                 name=f"a{lyr}{X}")
                    nc.vector.tensor_mul(av[:], self.rzf[lyr][:, 0:2, :],
                                         self.hn[:])
                    self.av[lyr] = av

                def utanh(self, t, lyr):
                    X = self.X
                    ug = psp.tile([128, 2, BH], F32, tag=f"u{X}",
                                  name=f"u{lyr}{X}")
                    for jg in range(2):
                        if lyr == 0:
                            id_mm(ug[:, jg, :], jg,
                                  hb(xg_tiles[t], X)[:, 4:6, :], True, False)
                        else:
                            _, h8p, hrp = self.h0_prev
                            mm(ug[:, jg, :],
                               wih1[:, :, (4 + jg) * 128:(5 + jg) * 128],
                               h8p, start=True, stop=False, perf_mode=DR)
                            mm(ug[:, jg, :],
                               wih1r[:, :, jg * 128:(jg + 1) * 128],
                               h8p, start=False, stop=False, perf_mode=DR)
                            mm(ug[:, jg, :],
                               wih1[:, :, (4 + jg) * 128:(5 + jg) * 128],
                               hrp, start=False, stop=False, perf_mode=DR)
                            brow_mm(ug[:, jg, :], BR_U1 + jg, X, stop=False)
                        # the a-injection last: everything independent of the
                        # DVE a-op has already streamed into the group
                        id_mm(ug[:, jg, :], jg, self.av[lyr][:], False, True)
                    nv = gp.tile([128, 2, BH], F16, tag=f"n{lyr}{X}",
                                 name=f"n{lyr}{X}")
                    act(nv[:], ug[:], AF.Tanh, scale=1.0 / SC)
                    self.nv[lyr] = nv

                def combine(self, t, lyr):
                    X = self.X
                    zfx = self.rzf[lyr][:, 2:4, :]
                    nv = self.nv[lyr]
                    h16x = (self.h0 if lyr == 0 else self.h1)[0]
                    d = gp.tile([128, 2, BH], F16, tag=f"d{lyr}{X}",
                                name=f"d{lyr}{X}")
                    nc.vector.tensor_sub(d[:], h16x, nv[:])
                    e = gp.tile([128, 2, BH], F16, tag=f"e{lyr}{X}",
                                name=f"e{lyr}{X}")
                    nc.vector.tensor_mul(e[:], d[:], zfx)
                    h16n = hp.tile([128, 2, BH], F16, tag=f"h{lyr}_16{X}",
                                   name=f"h{lyr}n16{X}")
                    nc.vector.tensor_add(h16n[:], e[:], nv[:])
                    h8n = hp.tile([128, 2, BH], F8, tag=f"h{lyr}_8{X}",
                                  name=f"h{lyr}n8{X}")
                    nc.gpsimd.tensor_copy(h8n[:], h16n[:])
                    hrn = hp.tile([128, 2, BH], F8, tag=f"h{lyr}_r{X}",
                                  name=f"h{lyr}nr{X}")
                    # residual: split across engines (DVE for layer 0, GpSimd
                    # for layer 1) to balance load
                    if lyr == 0:
                        nc.vector.tensor_sub(hrn[:], h16n[:], h8n[:])
                    else:
                        nc.gpsimd.tensor_sub(hrn[:], h16n[:], h8n[:])
                    hnew = (h16n, h8n, hrn)
                    if lyr == 0:
                        self.h0_prev = self.h0
                        self.h0 = hnew
                    else:
                        self.h1 = hnew

                def logits(self, t):
                    X = self.X
                    lo = psp.tile([128, 2, BH], F32, tag=f"u{X}", name="lo")
                    for vj in range(2):
                        mm(lo[:, vj, :], ident16[:],
                           ccpb[:, vj, X * BH:(X + 1) * BH],
                           start=True, stop=False)
                        for j in range(2):
                            mm(lo[:, vj, :],
                               proj[:, j, vj * 128:(vj + 1) * 128],
                               self.h1[0][:, j, :], start=False,
                               stop=(j == 1))
                    lo_sb = lp.tile([128, 2, BH], F16, tag=f"lo{X}",
                                    name=f"lo{X}")
                    act(lo_sb[:], lo[:], AF.Copy)
                    dma(out_d[t][:, :, X * BH:(X + 1) * BH], lo_sb[:])

            A, Bh = Half(0), Half(1)

            # 4-strand modulo schedule: per iteration t each half advances
            # its L0 strand to step t+1 and its L1 strand to step t (B's
            # slots sit half an iteration behind A's), so every engine's
            # in-order stream cycles through four independent chains.
            def stage(h, fn_name, t, lyr=None):
                if t < 0 or t >= T:
                    return
                f = getattr(h, fn_name)
                if lyr is None:
                    f(t)
                else:
                    f(t, lyr)

            for t in range(-1, T):
                if t + 3 < T:
                    fetch_xg(t + 3)
                stage(A, "sig", t + 1, 0)
                stage(Bh, "sig", t - 1, 1)
                stage(A, "amul", t + 1, 0)
                stage(Bh, "amul", t - 1, 1)
                stage(A, "utanh", t + 1, 0)
                stage(Bh, "utanh", t - 1, 1)
                if t + 1 < T:
                    A.combine(t + 1, 0)
                elif t + 1 == T:
                    A.h0_prev = A.h0
                stage(Bh, "combine", t - 1, 1)
                stage(Bh, "logits", t - 1)
                stage(A, "sig", t, 1)
                stage(Bh, "sig", t + 1, 0)
                stage(A, "amul", t, 1)
                stage(Bh, "amul", t + 1, 0)
                stage(A, "utanh", t, 1)
                stage(Bh, "utanh", t + 1, 0)
                stage(A, "combine", t, 1)
                stage(A, "logits", t)
                if t + 1 < T:
                    Bh.combine(t + 1, 0)
                elif t + 1 == T:
                    Bh.h0_prev = Bh.h0
                if 0 <= t - 1:
                    xg_tiles.pop(t - 1, None)

            stage(Bh, "sig", T - 1, 1)
            stage(Bh, "amul", T - 1, 1)
            stage(Bh, "utanh", T - 1, 1)
            stage(Bh, "combine", T - 1, 1)
            stage(Bh, "logits", T - 1)

    nc.compile()
    return nc


def _host_prep(inputs):
    """All-numpy precompute; returns per-core input maps."""
    f16 = np.float16
    eword = np.asarray(inputs["eword"], np.float32)
    target_ids = np.asarray(inputs["target_ids"])
    char_emb = np.asarray(inputs["char_emb"], np.float32)
    w_ih0 = np.asarray(inputs["gru_w_ih0"], np.float32)
    w_hh0 = np.asarray(inputs["gru_w_hh0"], np.float32)
    b_ih0 = np.asarray(inputs["gru_b_ih0"], np.float32)
    b_hh0 = np.asarray(inputs["gru_b_hh0"], np.float32)
    w_ih1 = np.asarray(inputs["gru_w_ih1"], np.float32)
    w_hh1 = np.asarray(inputs["gru_w_hh1"], np.float32)
    b_ih1 = np.asarray(inputs["gru_b_ih1"], np.float32)
    b_hh1 = np.asarray(inputs["gru_b_hh1"], np.float32)
    attn_in_w = np.asarray(inputs["attn_in_w"], np.float32)
    attn_in_b = np.asarray(inputs["attn_in_b"], np.float32)
    attn_out_w = np.asarray(inputs["attn_out_w"], np.float32)
    attn_out_b = np.asarray(inputs["attn_out_b"], np.float32)
    eword_proj_w = np.asarray(inputs["eword_proj_w"], np.float32)
    eword_proj_b = np.asarray(inputs["eword_proj_b"], np.float32)
    val_w = np.asarray(inputs["val_w"], np.float32)
    val_b = np.asarray(inputs["val_b"], np.float32)
    proj_w = np.asarray(inputs["proj_w"], np.float32)
    proj_b = np.asarray(inputs["proj_b"], np.float32)

    # x-side gate preactivations for all steps, L0 biases folded in
    in_ids = np.concatenate(
        [np.full((B, 1), BOS, target_ids.dtype), target_ids[:, :-1]], axis=1)
    cemb = char_emb.copy()
    cemb[PAD] = 0.0
    table = cemb @ w_ih0[:, :CE].T                      # (V, 3H)
    ge = eword @ w_ih0[:, CE:].T                        # (B, 3H)
    bias_vec = b_ih0.copy()
    bias_vec[:2 * H] += b_hh0[:2 * H]                   # r,z only
    xg = table[in_ids] + (ge + bias_vec)[:, None, :]    # (B, T, 3H)

    h0 = np.tanh(eword @ eword_proj_w.T + eword_proj_b)  # (B, H)

    wv = attn_in_w[2 * H:3 * H]
    bv = attn_in_b[2 * H:3 * H]
    ev = (eword @ val_w.T + val_b) @ wv.T + bv
    ao = ev @ attn_out_w.T + attn_out_b
    cc = ao @ proj_w.T + proj_b                          # (B, V)

    def pack_w(w):
        # (3H, H) -> lhsT [p_in, j_in, (g,jg)*128+p_out]
        w4 = w.reshape(3, 2, 128, 2, 128)   # g, jg_out, p_out, j_in, p_in
        return np.ascontiguousarray(
            w4.transpose(4, 3, 0, 1, 2).reshape(128, 2, 3 * H))

    def pack8(w):
        a = (SC * pack_w(w)).astype(NP_F8)
        r = ((SC * pack_w(w)) - a.astype(np.float32)).astype(NP_F8)
        return a, r[:, :, 2 * H:]          # residual kept for n-cols only

    whh0p, whh0r = pack8(w_hh0)
    wih1p, wih1r = pack8(w_ih1)
    whh1p, whh1r = pack8(w_hh1)
    shared = {
        "whh0p": whh0p, "whh0r": whh0r,
        "wih1p": wih1p, "wih1r": wih1r,
        "whh1p": whh1p, "whh1r": whh1r,
        "projp": np.ascontiguousarray(
            proj_w.reshape(2, 128, 2, 128).transpose(3, 2, 0, 1)
            .reshape(128, 2, V)).astype(f16),
        "ident16": np.eye(128, dtype=f16),
        "ones8": np.ones((1, 2, BP), np.float32).astype(NP_F8),
    }
    idr = np.zeros((128, 4, 128), np.float32)
    idr[:, 0, :] = np.eye(128)
    idr[:, 3, :] = np.eye(128)
    shared["identdr"] = idr.astype(NP_F8)

    brow = np.zeros((1, 2 * NBR, 128), np.float32)  # values stored at SC x
    b1 = b_ih1 + b_hh1
    for jg in range(2):
        brow[0, 2 * (BR_HN0 + jg)] = \
            b_hh0[2 * H + jg * 128: 2 * H + (jg + 1) * 128]
        brow[0, 2 * (BR_HN1 + jg)] = \
            b_hh1[2 * H + jg * 128: 2 * H + (jg + 1) * 128]
        brow[0, 2 * (BR_U1 + jg)] = \
            b_ih1[2 * H + jg * 128: 2 * H + (jg + 1) * 128]
    for s in range(4):
        g, jg = divmod(s, 2)
        brow[0, 2 * (BR_RZ1 + s)] = \
            b1[g * H + jg * 128: g * H + (jg + 1) * 128]
    shared["brow"] = (SC * brow).astype(NP_F8)

    def jmaj(x2d):
        # (BP, 256) -> [128, 2, BP]
        return np.ascontiguousarray(
            x2d.reshape(BP, 2, 128).transpose(2, 1, 0))

    def per_core(c):
        sl = slice(c * BP, (c + 1) * BP)
        m = dict(shared)
        hj = jmaj(h0[sl]).astype(f16)
        m["hinit16"] = hj
        h8 = hj.astype(np.float32).astype(NP_F8)
        m["hinit8"] = h8
        m["hinitr"] = (hj.astype(np.float32)
                       - h8.astype(np.float32)).astype(NP_F8)
        m["ccpb"] = jmaj(cc[sl]).astype(f16)
        # xg: (BP, T, 3H) -> [T, 128, (g,jg)6, BP], stored at SC x
        xc = xg[sl].reshape(BP, T, 3, 2, 128).transpose(1, 4, 2, 3, 0)
        m["xg"] = np.ascontiguousarray(
            SC * xc.reshape(T, 128, 6, BP)).astype(NP_F8)
        return m

    return [per_core(c) for c in range(NCORES)]


def kernel(**inputs):
    in_maps = _host_prep(inputs)
    if "nc" not in _CACHE:
        _CACHE["nc"] = _build_nc()
    nc = _CACHE["nc"]
    res = run_bass_kernel_spmd(nc, in_maps, list(range(NCORES)),
                               trace=bool(os.environ.get("BASS_TRACE")))
    _CACHE["last_res"] = res
    _CACHE["last_in_maps"] = in_maps
    outs = []
    for c in range(NCORES):
        o = res.results[c]["out"]            # (T, 128, 2, BP) f16
        outs.append(np.ascontiguousarray(
            o.transpose(3, 0, 2, 1).reshape(BP, T, V)).astype(np.float32))
    return np.concatenate(outs, axis=0)
